# revision 12
# baseline (speedup 1.0000x reference)
"""Single-head causal self-attention on 8 Trainium2 NeuronCores (Bass/Tile).

Problem: x [1024, 256, 384], Wq/Wk/Wv [384, 64] ->
  q,k,v = x@W;  wei = softmax(mask(q k^T / sqrt(384)));  out = wei @ v
Output: [1024, 256, 64] fp32.

v2 design (vs v1 baseline at 661us):
  - fp16 operands everywhere (err budget 2e-2; fp16 keeps ~1e-3). Halves
    DMA/SBUF traffic and enables fast-weight-load (FWL) on LDWEIGHTS.
  - 4 batches per iteration ("group"): one input DMA per group, N=512
    moving dims on the projection matmuls, far fewer instructions.
  - V computed directly in [t, h] layout using x^T chunks as the
    stationary operand (no PE transposes, no eye matrix).
  - Softmax denominator via the ones-column trick (row 64 of outT);
    normalization (divide by denom) moved to the host gather step, which
    kills the 1-partition DVE reciprocal (1.75us each!), the K=1
    broadcast matmul, and two more per-batch ops.
  - Software-pipelined issue order across 3 groups so the PE never
    idles: HAM throttle (PE at 1.2GHz for 96% of v1) stays released.

Per-core layout (128 batches = 32 groups of 4):
  xt4 [128, 3, 1024] f16 per group: xt4[p, c, 256*i + t] = x[4g+i, t, 128c+p]
  qk MMs:   [Wq|Wk]_c^T @ xt4 pair-half -> psQK [q(0:64); k(64:128), 512]
  v MMs:    xt4_chunk(t-half)^T @ Wv_c  -> psV [128(t), 64] per b,half
  weiT MMs: k2^T(s-half) q2 (K=64)      -> psW [128(s-half), 256(t)] x2
  P = exp(weiT*scale) (ACT, fp16 out), tril mask on diag blocks (DVE)
  outT MMs: [v|1]^T P (K=128, s-halves) -> psO [65, 256] (row 64 = denom)
  out DMA [65, 1024] f16 per group; host: out = outT[0:64]/outT[64], transpose.
"""

import os
from contextlib import ExitStack

import numpy as np

import concourse.bass as bass
import concourse.bacc as bacc
import concourse.tile as tile
from concourse import mybir
from concourse.bass_utils import run_bass_kernel_spmd

N_CORES = 8
B = 1024
T = 256
C = 384
H = 64
BPC = B // N_CORES  # 128 batches per core
GRP = 4  # batches per group
NG = BPC // GRP  # 32 groups per core
NCHUNK = C // 128  # 3
SCALE = float(C) ** -0.5

F32 = mybir.dt.float32
F16 = mybir.dt.float16
Exp = mybir.ActivationFunctionType.Exp


def build_nc(ng: int = NG):
    nc = bacc.Bacc(
        "TRN2", target_bir_lowering=False, debug=False, num_devices=N_CORES
    )

    xt = nc.dram_tensor("xt", [ng, 128, NCHUNK, GRP * T], F16, kind="ExternalInput").ap()
    wqk = nc.dram_tensor("wqk", [128, NCHUNK, 128], F16, kind="ExternalInput").ap()
    wv = nc.dram_tensor("wv", [128, NCHUNK, H], F16, kind="ExternalInput").ap()
    mask = nc.dram_tensor("mask", [128, 2 * 128], F16, kind="ExternalInput").ap()
    outT = nc.dram_tensor("outT", [ng, H + 1, GRP * T], F16, kind="ExternalOutput").ap()

    with ExitStack() as ctx:
        tc = ctx.enter_context(tile.TileContext(nc))

        const = ctx.enter_context(tc.tile_pool(name="const", bufs=1))
        wqk_sb = const.tile([128, NCHUNK, 128], F16, tag="wqk")
        nc.sync.dma_start(wqk_sb[:], wqk)
        wv_sb = const.tile([128, NCHUNK, H], F16, tag="wv")
        nc.sync.dma_start(wv_sb[:], wv)
        mask_sb = const.tile([128, 2 * 128], F16, tag="mask")
        nc.sync.dma_start(mask_sb[:], mask)

        # Persistent tiles: v_aug ones-columns (64 and 129 per batch slot).
        vaug = []
        for i in range(2):
            v_t = const.tile([128, GRP, 131], F16, tag=f"vaug{i}")
            nc.gpsimd.memset(v_t[:, :, 64:65], 1.0)
            nc.gpsimd.memset(v_t[:, :, 129:130], 1.0)
            vaug.append(v_t)

        xt_pool = ctx.enter_context(tc.tile_pool(name="xt", bufs=3))
        qk_pool = ctx.enter_context(tc.tile_pool(name="qk", bufs=4))
        p0_pool = ctx.enter_context(tc.tile_pool(name="p0", bufs=4))
        o_pool = ctx.enter_context(tc.tile_pool(name="o", bufs=2))
        psqk_pool = ctx.enter_context(tc.tile_pool(name="psqk", bufs=2, space="PSUM"))
        psv_pool = ctx.enter_context(tc.tile_pool(name="psv", bufs=2, space="PSUM"))
        psw_pool = ctx.enter_context(tc.tile_pool(name="psw", bufs=2, space="PSUM"))
        pso_pool = ctx.enter_context(tc.tile_pool(name="pso", bufs=2, space="PSUM"))

        # Per-group state carried between pipeline stages.
        st = {}

        for i in range(ng + 2):
            g0 = i  # stage 0: input DMA
            g1 = i - 1  # stage 1: qk + v matmuls and evacuations
            g2 = i - 2  # stage 2: weiT, softmax, outT, output DMA

            if g0 < ng:
                xt_sb = xt_pool.tile([128, NCHUNK, GRP * T], F16, tag="xt")
                nc.sync.dma_start(xt_sb[:], xt[g0])
                st[g0] = {"xt": xt_sb}

            if 0 <= g2:
                s2 = st[g2]
                # weiT per batch into psw [128, 384]:
                #   cols 0:128   = wei[s 128:256, t 128:256] (s-half1 diag)
                #   cols 128:384 = wei[s 0:128,   t 0:256]   (s-half0 full)
                # The all-masked (s-half1, t<128) block is never computed.
                # Pair A (batches 0,1) lives at partitions 0:64 and runs on
                # PE rows 0:64; pair B (batches 2,3) at partitions 64:128 on
                # rows 64:128 — the two pairs' matmuls execute concurrently.
                q4, k4 = s2["q4"], s2["k4"]
                psw = {}
                for w in range(2):  # wave: batch within pair
                    for mm in range(2):
                        for j in range(2):  # pair
                            b = 2 * j + w
                            if mm == 0:
                                psw[b] = psw_pool.tile(
                                    [128, 384], F32, tag="psw", name="psw"
                                )
                                nc.tensor.matmul(
                                    psw[b][:, 0:128],
                                    lhsT=k4[64 * j : 64 * (j + 1), w * T + 128 : (w + 1) * T],
                                    rhs=q4[64 * j : 64 * (j + 1), w * T + 128 : (w + 1) * T],
                                    start=True,
                                    stop=True,
                                    tile_position=(64 * j, 0),
                                )
                            else:
                                nc.tensor.matmul(
                                    psw[b][:, 128:384],
                                    lhsT=k4[64 * j : 64 * (j + 1), w * T : w * T + 128],
                                    rhs=q4[64 * j : 64 * (j + 1), w * T : (w + 1) * T],
                                    start=True,
                                    stop=True,
                                    tile_position=(64 * j, 0),
                                )
                s2["psw"] = psw

                # exp + mask for wave-0 batches (0, 2) — chase the weiT MMs
                s2["p0"] = {}
                for b in (0, 2):
                    p0 = p0_pool.tile([128, 384], F16, tag="p0")
                    nc.scalar.activation(p0[:], s2["psw"][b][:], Exp, scale=SCALE)
                    nc.gpsimd.tensor_mul(p0[:, 0:256], p0[:, 0:256], mask_sb[:])
                    s2["p0"][b] = p0

            if 0 <= g1 < ng:
                s1 = st[g1]
                xt_sb = s1["xt"]
                # qk: per pair j, 3 accumulating MMs N=512 -> [q;k] stacked.
                # Evacuate pair A to partitions 0:64 of q4/k4, pair B to
                # partitions 64:128 (feeds the row-packed weiT above).
                q4 = qk_pool.tile([128, 512], F16, tag="q4")
                k4 = qk_pool.tile([128, 512], F16, tag="k4")
                for j in range(2):
                    psqk = psqk_pool.tile([128, 512], F32, tag="psqk")
                    for c in range(NCHUNK):
                        nc.tensor.matmul(
                            psqk[:],
                            lhsT=wqk_sb[:, c, :],
                            rhs=xt_sb[:, c, j * 512 : (j + 1) * 512],
                            start=(c == 0),
                            stop=(c == NCHUNK - 1),
                        )
                    nc.scalar.copy(q4[64 * j : 64 * (j + 1), :], psqk[0:64, :])
                    nc.vector.tensor_copy(
                        k4[64 * j : 64 * (j + 1), :], psqk[64:128, :]
                    )
                s1["q4"], s1["k4"] = q4, k4

            if 0 <= g2:
                s2 = st[g2]
                # exp + mask for wave-1 batches (1, 3)
                for b in (1, 3):
                    p0 = p0_pool.tile([128, 384], F16, tag="p0")
                    nc.scalar.activation(p0[:], s2["psw"][b][:], Exp, scale=SCALE)
                    nc.gpsimd.tensor_mul(p0[:, 0:256], p0[:, 0:256], mask_sb[:])
                    s2["p0"][b] = p0

            if 0 <= g1 < ng:
                s1 = st[g1]
                xt_sb = s1["xt"]
                # v direct in [t, h] layout: stationary = xt chunk t-half,
                # moving = Wv chunk. 8 regions x 3 accumulating MMs, N=64.
                psv = psv_pool.tile([128, 512], F32, tag="psv")
                for b in range(GRP):
                    for th in range(2):
                        off = b * 128 + th * 64
                        toff = b * T + th * 128
                        for c in range(NCHUNK):
                            nc.tensor.matmul(
                                psv[:, off : off + 64],
                                lhsT=xt_sb[:, c, toff : toff + 128],
                                rhs=wv_sb[:, c, :],
                                start=(c == 0),
                                stop=(c == NCHUNK - 1),
                            )
                v4 = vaug[g1 % 2]
                dst = v4[:, :, 0:130].rearrange(
                    "p b (two f) -> p b two f", two=2
                )[:, :, :, 0:64]
                src = psv[:].rearrange("p (b two f) -> p b two f", b=GRP, two=2)
                nc.vector.tensor_copy(dst, src)
                s1["v4"] = v4

            if 0 <= g2:
                s2 = st[g2]
                v4 = s2["v4"]
                # outT: per batch, 2 accumulating MMs (s-halves), M=65
                # (col 64 of v_aug is ones -> row 64 = softmax denom).
                s2["pso"] = []
                for j in range(2):
                    pso = pso_pool.tile([65, 512], F32, tag="pso")
                    for bb in range(2):
                        b = j * 2 + bb
                        nc.tensor.matmul(
                            pso[:, bb * T : (bb + 1) * T],
                            lhsT=v4[:, b, 0:65],
                            rhs=s2["p0"][b][:, 128:384],
                            start=True,
                            stop=False,
                        )
                        # s-half1 contributes only to t >= 128 (causal).
                        nc.tensor.matmul(
                            pso[:, bb * T + 128 : (bb + 1) * T],
                            lhsT=v4[:, b, 65:130],
                            rhs=s2["p0"][b][:, 0:128],
                            start=False,
                            stop=True,
                        )
                    s2["pso"].append(pso)

                o_sb = o_pool.tile([H + 1, GRP * T], F16, tag="o")
                nc.vector.tensor_copy(o_sb[:, 0 : 2 * T], s2["pso"][0][:])
                nc.vector.tensor_copy(o_sb[:, 2 * T : 4 * T], s2["pso"][1][:])
                nc.sync.dma_start(outT[g2], o_sb[:])
                del st[g2]

    nc.finalize()
    return nc


def _host_inputs(x, Wq, Wk, Wv):
    B_, T_, C_ = x.shape
    assert (B_, T_, C_) == (B, T, C), (B_, T_, C_)
    # xh[g, p, c, 256*i + t] = x[4g+i, t, 128c+p]
    xh = np.ascontiguousarray(
        x.reshape(B // GRP, GRP, T, NCHUNK, 128)
        .transpose(0, 4, 3, 1, 2)
        .reshape(B // GRP, 128, NCHUNK, GRP * T)
        .astype(np.float16)
    )
    wqk_h = np.ascontiguousarray(
        np.concatenate([Wq, Wk], axis=1).reshape(NCHUNK, 128, 128).transpose(1, 0, 2),
        dtype=np.float16,
    )
    wv_h = np.ascontiguousarray(
        Wv.reshape(NCHUNK, 128, H).transpose(1, 0, 2), dtype=np.float16
    )
    tri = np.triu(np.ones((128, 128), dtype=np.float16))
    mask_h = np.ascontiguousarray(np.concatenate([tri, tri], axis=1))
    return xh, wqk_h, wv_h, mask_h


def _gather(results):
    """Concatenate per-core outT, normalize, and restore [B, T, H] fp32."""
    outT = np.concatenate(
        [results[i]["outT"] for i in range(N_CORES)], axis=0
    ).astype(np.float32)  # [B/GRP, 65, GRP*T]
    outT = outT.reshape(B // GRP, H + 1, GRP, T)
    out = outT[:, 0:H] / outT[:, H : H + 1]  # [B/GRP, H, GRP, T]
    return np.ascontiguousarray(
        out.transpose(0, 2, 3, 1).reshape(B, T, H).astype(np.float32)
    )


def kernel(x, Wq, Wk, Wv):
    x = np.asarray(x, dtype=np.float32)
    Wq = np.asarray(Wq, dtype=np.float32)
    Wk = np.asarray(Wk, dtype=np.float32)
    Wv = np.asarray(Wv, dtype=np.float32)

    xh, wqk_h, wv_h, mask_h = _host_inputs(x, Wq, Wk, Wv)

    nc = build_nc(NG)
    in_maps = [
        {
            "xt": xh[i * NG : (i + 1) * NG],
            "wqk": wqk_h,
            "wv": wv_h,
            "mask": mask_h,
        }
        for i in range(N_CORES)
    ]
    res = run_bass_kernel_spmd(nc, in_maps, list(range(N_CORES)))
    return _gather(res.results)


# revision 14
# speedup vs baseline: 1.0280x; 1.0280x over previous
"""Single-head causal self-attention on 8 Trainium2 NeuronCores (Bass/Tile).

Problem: x [1024, 256, 384], Wq/Wk/Wv [384, 64] ->
  q,k,v = x@W;  wei = softmax(mask(q k^T / sqrt(384)));  out = wei @ v
Output: [1024, 256, 64] fp32.

v2 design (vs v1 baseline at 661us):
  - fp16 operands everywhere (err budget 2e-2; fp16 keeps ~1e-3). Halves
    DMA/SBUF traffic and enables fast-weight-load (FWL) on LDWEIGHTS.
  - 4 batches per iteration ("group"): one input DMA per group, N=512
    moving dims on the projection matmuls, far fewer instructions.
  - V computed directly in [t, h] layout using x^T chunks as the
    stationary operand (no PE transposes, no eye matrix).
  - Softmax denominator via the ones-column trick (row 64 of outT);
    normalization (divide by denom) moved to the host gather step, which
    kills the 1-partition DVE reciprocal (1.75us each!), the K=1
    broadcast matmul, and two more per-batch ops.
  - Software-pipelined issue order across 3 groups so the PE never
    idles: HAM throttle (PE at 1.2GHz for 96% of v1) stays released.

Per-core layout (128 batches = 32 groups of 4):
  xt4 [128, 3, 1024] f16 per group: xt4[p, c, 256*i + t] = x[4g+i, t, 128c+p]
  qk MMs:   [Wq|Wk]_c^T @ xt4 pair-half -> psQK [q(0:64); k(64:128), 512]
  v MMs:    xt4_chunk(t-half)^T @ Wv_c  -> psV [128(t), 64] per b,half
  weiT MMs: k2^T(s-half) q2 (K=64)      -> psW [128(s-half), 256(t)] x2
  P = exp(weiT*scale) (ACT, fp16 out), tril mask on diag blocks (DVE)
  outT MMs: [v|1]^T P (K=128, s-halves) -> psO [65, 256] (row 64 = denom)
  out DMA [65, 1024] f16 per group; host: out = outT[0:64]/outT[64], transpose.
"""

import os
from contextlib import ExitStack

import numpy as np

import concourse.bass as bass
import concourse.bacc as bacc
import concourse.tile as tile
from concourse import mybir
from concourse.bass_utils import run_bass_kernel_spmd

N_CORES = 8
B = 1024
T = 256
C = 384
H = 64
BPC = B // N_CORES  # 128 batches per core
GRP = 4  # batches per group
NG = BPC // GRP  # 32 groups per core
NCHUNK = C // 128  # 3
SCALE = float(C) ** -0.5

F32 = mybir.dt.float32
F16 = mybir.dt.float16
Exp = mybir.ActivationFunctionType.Exp


def build_nc(ng: int = NG):
    nc = bacc.Bacc(
        "TRN2", target_bir_lowering=False, debug=False, num_devices=N_CORES
    )

    xt = nc.dram_tensor("xt", [ng, 128, NCHUNK, GRP * T], F16, kind="ExternalInput").ap()
    wqk = nc.dram_tensor("wqk", [128, NCHUNK, 128], F16, kind="ExternalInput").ap()
    wv = nc.dram_tensor("wv", [128, NCHUNK, H], F16, kind="ExternalInput").ap()
    mask = nc.dram_tensor("mask", [128, 2 * 128], F16, kind="ExternalInput").ap()
    outT = nc.dram_tensor("outT", [ng, H + 1, GRP * T], F16, kind="ExternalOutput").ap()

    with ExitStack() as ctx:
        tc = ctx.enter_context(tile.TileContext(nc))

        const = ctx.enter_context(tc.tile_pool(name="const", bufs=1))
        wqk_sb = const.tile([128, NCHUNK, 128], F16, tag="wqk")
        nc.sync.dma_start(wqk_sb[:], wqk)
        wv_sb = const.tile([128, NCHUNK, H], F16, tag="wv")
        nc.sync.dma_start(wv_sb[:], wv)
        mask_sb = const.tile([128, 2 * 128], F16, tag="mask")
        nc.sync.dma_start(mask_sb[:], mask)

        # Persistent tiles: v_aug ones-columns (64 and 129 per batch slot).
        vaug = []
        for i in range(2):
            v_t = const.tile([128, GRP, 131], F16, tag=f"vaug{i}")
            nc.gpsimd.memset(v_t[:, :, 64:65], 1.0)
            nc.gpsimd.memset(v_t[:, :, 129:130], 1.0)
            vaug.append(v_t)

        xt_pool = ctx.enter_context(tc.tile_pool(name="xt", bufs=3))
        qk_pool = ctx.enter_context(tc.tile_pool(name="qk", bufs=2))
        p0_pool = ctx.enter_context(tc.tile_pool(name="p0", bufs=8))
        o_pool = ctx.enter_context(tc.tile_pool(name="o", bufs=2))
        psqk_pool = ctx.enter_context(tc.tile_pool(name="psqk", bufs=2, space="PSUM"))
        psv_pool = ctx.enter_context(tc.tile_pool(name="psv", bufs=2, space="PSUM"))
        psw_pool = ctx.enter_context(tc.tile_pool(name="psw", bufs=2, space="PSUM"))
        pso_pool = ctx.enter_context(tc.tile_pool(name="pso", bufs=2, space="PSUM"))

        # Per-group state carried between pipeline stages.
        st = {}

        for i in range(ng + 2):
            g0 = i  # stage 0: input DMA
            g1 = i - 1  # stage 1: qk/v matmuls, evacuations, weiT, softmax
            g2 = i - 2  # stage 2: outT, output evac + DMA

            if g0 < ng:
                xt_sb = xt_pool.tile([128, NCHUNK, GRP * T], F16, tag="xt")
                nc.sync.dma_start(xt_sb[:], xt[g0])
                st[g0] = {"xt": xt_sb}

            if 0 <= g1 < ng:
                s1 = st[g1]
                xt_sb = s1["xt"]
                # qk: per pair j, 3 accumulating MMs N=512 -> [q;k] stacked.
                # Evacuate pair A to partitions 0:64 of q4/k4, pair B to
                # partitions 64:128 (feeds the row-packed weiT below).
                q4 = qk_pool.tile([128, 512], F16, tag="q4")
                k4 = qk_pool.tile([128, 512], F16, tag="k4")
                for j in range(2):
                    psqk = psqk_pool.tile([128, 512], F32, tag="psqk")
                    for c in range(NCHUNK):
                        nc.tensor.matmul(
                            psqk[:],
                            lhsT=wqk_sb[:, c, :],
                            rhs=xt_sb[:, c, j * 512 : (j + 1) * 512],
                            start=(c == 0),
                            stop=(c == NCHUNK - 1),
                        )
                    nc.scalar.copy(q4[64 * j : 64 * (j + 1), :], psqk[0:64, :])
                    nc.vector.tensor_copy(
                        k4[64 * j : 64 * (j + 1), :], psqk[64:128, :]
                    )
                s1["q4"], s1["k4"] = q4, k4

                # v direct in [t, h] layout: stationary = xt chunk t-half,
                # moving = Wv chunk. 8 regions x 3 accumulating MMs, N=64.
                psv = psv_pool.tile([128, 512], F32, tag="psv")
                for b in range(GRP):
                    for th in range(2):
                        off = b * 128 + th * 64
                        toff = b * T + th * 128
                        for c in range(NCHUNK):
                            nc.tensor.matmul(
                                psv[:, off : off + 64],
                                lhsT=xt_sb[:, c, toff : toff + 128],
                                rhs=wv_sb[:, c, :],
                                start=(c == 0),
                                stop=(c == NCHUNK - 1),
                            )
                v4 = vaug[g1 % 2]
                dst = v4[:, :, 0:130].rearrange(
                    "p b (two f) -> p b two f", two=2
                )[:, :, :, 0:64]
                src = psv[:].rearrange("p (b two f) -> p b two f", b=GRP, two=2)
                nc.vector.tensor_copy(dst, src)
                s1["v4"] = v4

            if 0 <= g2:
                s2 = st[g2]
                v4 = s2["v4"]
                # outT: per batch, 2 accumulating MMs (s-halves), M=65
                # (col 64 of v_aug is ones -> row 64 = softmax denom).
                # P tiles were produced last iteration, so no PE stall here;
                # this also fills the wait for this iteration's q/k evacs.
                pso = [
                    pso_pool.tile([65, 512], F32, tag="pso", name="pso")
                    for _ in range(2)
                ]
                for b in (0, 2, 1, 3):
                    j, bb = divmod(b, 2)
                    nc.tensor.matmul(
                        pso[j][:, bb * T : (bb + 1) * T],
                        lhsT=v4[:, b, 0:65],
                        rhs=s2["p0"][b][:, 128:384],
                        start=True,
                        stop=False,
                    )
                    # s-half1 contributes only to t >= 128 (causal).
                    nc.tensor.matmul(
                        pso[j][:, bb * T + 128 : (bb + 1) * T],
                        lhsT=v4[:, b, 65:130],
                        rhs=s2["p0"][b][:, 0:128],
                        start=False,
                        stop=True,
                    )

                o_sb = o_pool.tile([H + 1, GRP * T], F16, tag="o")
                nc.vector.tensor_copy(o_sb[:, 0 : 2 * T], pso[0][:])
                nc.vector.tensor_copy(o_sb[:, 2 * T : 4 * T], pso[1][:])
                nc.sync.dma_start(outT[g2], o_sb[:])
                del st[g2]

            if 0 <= g1 < ng:
                s1 = st[g1]
                q4, k4 = s1["q4"], s1["k4"]
                # weiT per batch into psw [128, 384]:
                #   cols 0:128   = wei[s 128:256, t 128:256] (s-half1 diag)
                #   cols 128:384 = wei[s 0:128,   t 0:256]   (s-half0 full)
                # The all-masked (s-half1, t<128) block is never computed.
                # Pair A (batches 0,1) lives at partitions 0:64 and runs on
                # PE rows 0:64; pair B (batches 2,3) at partitions 64:128 on
                # rows 64:128 — the two pairs' matmuls execute concurrently.
                psw = {}
                for w in range(2):  # wave: batch within pair
                    for mm in range(2):
                        for j in range(2):  # pair
                            b = 2 * j + w
                            if mm == 0:
                                psw[b] = psw_pool.tile(
                                    [128, 384], F32, tag="psw", name="psw"
                                )
                                nc.tensor.matmul(
                                    psw[b][:, 0:128],
                                    lhsT=k4[64 * j : 64 * (j + 1), w * T + 128 : (w + 1) * T],
                                    rhs=q4[64 * j : 64 * (j + 1), w * T + 128 : (w + 1) * T],
                                    start=True,
                                    stop=True,
                                    tile_position=(64 * j, 0),
                                )
                            else:
                                nc.tensor.matmul(
                                    psw[b][:, 128:384],
                                    lhsT=k4[64 * j : 64 * (j + 1), w * T : w * T + 128],
                                    rhs=q4[64 * j : 64 * (j + 1), w * T : (w + 1) * T],
                                    start=True,
                                    stop=True,
                                    tile_position=(64 * j, 0),
                                )

                # exp + mask (P consumed by outT next iteration)
                s1["p0"] = {}
                for b in (0, 2, 1, 3):
                    p0 = p0_pool.tile([128, 384], F16, tag="p0")
                    nc.scalar.activation(p0[:], psw[b][:], Exp, scale=SCALE)
                    nc.gpsimd.tensor_mul(p0[:, 0:256], p0[:, 0:256], mask_sb[:])
                    s1["p0"][b] = p0

    nc.finalize()
    return nc


def _host_inputs(x, Wq, Wk, Wv):
    B_, T_, C_ = x.shape
    assert (B_, T_, C_) == (B, T, C), (B_, T_, C_)
    # xh[g, p, c, 256*i + t] = x[4g+i, t, 128c+p]
    xh = np.ascontiguousarray(
        x.reshape(B // GRP, GRP, T, NCHUNK, 128)
        .transpose(0, 4, 3, 1, 2)
        .reshape(B // GRP, 128, NCHUNK, GRP * T)
        .astype(np.float16)
    )
    wqk_h = np.ascontiguousarray(
        np.concatenate([Wq, Wk], axis=1).reshape(NCHUNK, 128, 128).transpose(1, 0, 2),
        dtype=np.float16,
    )
    wv_h = np.ascontiguousarray(
        Wv.reshape(NCHUNK, 128, H).transpose(1, 0, 2), dtype=np.float16
    )
    tri = np.triu(np.ones((128, 128), dtype=np.float16))
    mask_h = np.ascontiguousarray(np.concatenate([tri, tri], axis=1))
    return xh, wqk_h, wv_h, mask_h


def _gather(results):
    """Concatenate per-core outT, normalize, and restore [B, T, H] fp32."""
    outT = np.concatenate(
        [results[i]["outT"] for i in range(N_CORES)], axis=0
    ).astype(np.float32)  # [B/GRP, 65, GRP*T]
    outT = outT.reshape(B // GRP, H + 1, GRP, T)
    out = outT[:, 0:H] / outT[:, H : H + 1]  # [B/GRP, H, GRP, T]
    return np.ascontiguousarray(
        out.transpose(0, 2, 3, 1).reshape(B, T, H).astype(np.float32)
    )


def kernel(x, Wq, Wk, Wv):
    x = np.asarray(x, dtype=np.float32)
    Wq = np.asarray(Wq, dtype=np.float32)
    Wk = np.asarray(Wk, dtype=np.float32)
    Wv = np.asarray(Wv, dtype=np.float32)

    xh, wqk_h, wv_h, mask_h = _host_inputs(x, Wq, Wk, Wv)

    nc = build_nc(NG)
    in_maps = [
        {
            "xt": xh[i * NG : (i + 1) * NG],
            "wqk": wqk_h,
            "wv": wv_h,
            "mask": mask_h,
        }
        for i in range(N_CORES)
    ]
    res = run_bass_kernel_spmd(nc, in_maps, list(range(N_CORES)))
    return _gather(res.results)


# revision 19
# speedup vs baseline: 1.0379x; 1.0096x over previous
"""Single-head causal self-attention on 8 Trainium2 NeuronCores (Bass/Tile).

Problem: x [1024, 256, 384], Wq/Wk/Wv [384, 64] ->
  q,k,v = x@W;  wei = softmax(mask(q k^T / sqrt(384)));  out = wei @ v
Output: [1024, 256, 64] fp32.

v2 design (vs v1 baseline at 661us):
  - fp16 operands everywhere (err budget 2e-2; fp16 keeps ~1e-3). Halves
    DMA/SBUF traffic and enables fast-weight-load (FWL) on LDWEIGHTS.
  - 4 batches per iteration ("group"): one input DMA per group, N=512
    moving dims on the projection matmuls, far fewer instructions.
  - V computed directly in [t, h] layout using x^T chunks as the
    stationary operand (no PE transposes, no eye matrix).
  - Softmax denominator via the ones-column trick (row 64 of outT);
    normalization (divide by denom) moved to the host gather step, which
    kills the 1-partition DVE reciprocal (1.75us each!), the K=1
    broadcast matmul, and two more per-batch ops.
  - Software-pipelined issue order across 3 groups so the PE never
    idles: HAM throttle (PE at 1.2GHz for 96% of v1) stays released.

Per-core layout (128 batches = 32 groups of 4):
  xt4 [128, 3, 1024] f16 per group: xt4[p, c, 256*i + t] = x[4g+i, t, 128c+p]
  qk MMs:   [Wq|Wk]_c^T @ xt4 pair-half -> psQK [q(0:64); k(64:128), 512]
  v MMs:    xt4_chunk(t-half)^T @ Wv_c  -> psV [128(t), 64] per b,half
  weiT MMs: k2^T(s-half) q2 (K=64)      -> psW [128(s-half), 256(t)] x2
  P = exp(weiT*scale) (ACT, fp16 out), tril mask on diag blocks (DVE)
  outT MMs: [v|1]^T P (K=128, s-halves) -> psO [65, 256] (row 64 = denom)
  out DMA [65, 1024] f16 per group; host: out = outT[0:64]/outT[64], transpose.
"""

import os
from contextlib import ExitStack

import numpy as np

import concourse.bass as bass
import concourse.bacc as bacc
import concourse.tile as tile
from concourse import mybir
from concourse.bass_utils import run_bass_kernel_spmd

N_CORES = 8
B = 1024
T = 256
C = 384
H = 64
BPC = B // N_CORES  # 128 batches per core
GRP = 4  # batches per group
NG = BPC // GRP  # 32 groups per core
NCHUNK = C // 128  # 3
SCALE = float(C) ** -0.5

F32 = mybir.dt.float32
F16 = mybir.dt.float16
Exp = mybir.ActivationFunctionType.Exp


def _weiT(nc, psw_pool, s1, qz, kz, b):
    """wei^T for batch b via K=128 zero-padded matmuls into psw [128, 384]:
    cols 0:128 = wei[s 128:256, t 128:256], cols 128:384 = wei[s 0:128, t]."""
    j, bb = divmod(b, 2)
    base = j * 512 + bb * T
    psw = psw_pool.tile([128, 384], mybir.dt.float32, tag="psw", name="psw")
    nc.tensor.matmul(
        psw[:, 0:128],
        lhsT=kz[:, base + 128 : base + T],
        rhs=qz[:, base + 128 : base + T],
        start=True,
        stop=True,
    )
    nc.tensor.matmul(
        psw[:, 128:384],
        lhsT=kz[:, base : base + 128],
        rhs=qz[:, base : base + T],
        start=True,
        stop=True,
    )
    s1["psw"][b] = psw


def _softmax(nc, p0_pool, s1, mask_sb, b):
    p0 = p0_pool.tile([128, 384], F16, tag="p0", name="p0")
    nc.scalar.activation(p0[:], s1["psw"][b][:], Exp, scale=SCALE)
    nc.gpsimd.tensor_mul(p0[:, 0:256], p0[:, 0:256], mask_sb[:])
    s1["p0"][b] = p0


def build_nc(ng: int = NG):
    nc = bacc.Bacc(
        "TRN2", target_bir_lowering=False, debug=False, num_devices=N_CORES
    )

    xt = nc.dram_tensor("xt", [ng, 128, NCHUNK, GRP * T], F16, kind="ExternalInput").ap()
    wqk = nc.dram_tensor("wqk", [128, NCHUNK, 128], F16, kind="ExternalInput").ap()
    wv = nc.dram_tensor("wv", [128, NCHUNK, H], F16, kind="ExternalInput").ap()
    mask = nc.dram_tensor("mask", [128, 2 * 128], F16, kind="ExternalInput").ap()
    outT = nc.dram_tensor("outT", [ng, H + 1, GRP * T], F16, kind="ExternalOutput").ap()

    with ExitStack() as ctx:
        tc = ctx.enter_context(tile.TileContext(nc))

        const = ctx.enter_context(tc.tile_pool(name="const", bufs=1))
        wqk_sb = const.tile([128, NCHUNK, 128], F16, tag="wqk")
        nc.sync.dma_start(wqk_sb[:], wqk)
        wv_sb = const.tile([128, NCHUNK, H], F16, tag="wv")
        nc.sync.dma_start(wv_sb[:], wv)
        mask_sb = const.tile([128, 2 * 128], F16, tag="mask")
        nc.sync.dma_start(mask_sb[:], mask)

        # Persistent tiles.
        # v_aug slots: per batch [128, 256]: cols 0:64 = v s-half0, col 64 =
        # ones (denominator row), cols 65:128 = ZERO pad (so the outT
        # LDWEIGHTS is a 128-column load -> fast-weight-load kicks in),
        # cols 128:192 = v s-half1, col 192 = ones, 193:256 = zero pad.
        vaug = []
        for i in range(2):
            v_t = const.tile([128, GRP, 256], F16, tag=f"vaug{i}")
            nc.gpsimd.memset(v_t[:], 0.0)
            nc.gpsimd.memset(v_t[:, :, 64:65], 1.0)
            nc.gpsimd.memset(v_t[:, :, 192:193], 1.0)
            vaug.append(v_t)
        # q/k slots for the K=128 zero-padded weiT: rows 0:64 hold q (k),
        # rows 64:128 stay zero so the padded contraction adds nothing.
        qzs, kzs = [], []
        for i in range(2):
            q_t = const.tile([128, 1024], F16, tag=f"qz{i}")
            nc.gpsimd.memset(q_t[64:128, :], 0.0)
            qzs.append(q_t)
            k_t = const.tile([128, 1024], F16, tag=f"kz{i}")
            nc.gpsimd.memset(k_t[64:128, :], 0.0)
            kzs.append(k_t)

        xt_pool = ctx.enter_context(tc.tile_pool(name="xt", bufs=3))
        p0_pool = ctx.enter_context(tc.tile_pool(name="p0", bufs=8))
        o_pool = ctx.enter_context(tc.tile_pool(name="o", bufs=2))
        psqk_pool = ctx.enter_context(tc.tile_pool(name="psqk", bufs=2, space="PSUM"))
        psv_pool = ctx.enter_context(tc.tile_pool(name="psv", bufs=2, space="PSUM"))
        psw_pool = ctx.enter_context(tc.tile_pool(name="psw", bufs=2, space="PSUM"))
        pso_pool = ctx.enter_context(tc.tile_pool(name="pso", bufs=2, space="PSUM"))

        # Per-group state carried between pipeline stages.
        st = {}

        for i in range(ng + 2):
            g0 = i  # stage 0: input DMA
            g1 = i - 1  # stage 1: qk/v matmuls, evacuations, weiT, softmax
            g2 = i - 2  # stage 2: outT, output evac + DMA

            if g0 < ng:
                xt_sb = xt_pool.tile([128, NCHUNK, GRP * T], F16, tag="xt")
                nc.sync.dma_start(xt_sb[:], xt[g0])
                st[g0] = {"xt": xt_sb}

            if 0 <= g1 < ng:
                s1 = st[g1]
                xt_sb = s1["xt"]
                # qk: per pair j, 3 accumulating MMs N=512 -> [q;k] stacked.
                # Evacuate q to rows 0:64 of the qz slot (cols j*512+),
                # k likewise into kz; rows 64:128 are persistent zeros.
                qz, kz = qzs[g1 % 2], kzs[g1 % 2]
                for j in range(2):
                    psqk = psqk_pool.tile([128, 512], F32, tag="psqk")
                    for c in range(NCHUNK):
                        nc.tensor.matmul(
                            psqk[:],
                            lhsT=wqk_sb[:, c, :],
                            rhs=xt_sb[:, c, j * 512 : (j + 1) * 512],
                            start=(c == 0),
                            stop=(c == NCHUNK - 1),
                        )
                    nc.scalar.copy(
                        qz[0:64, j * 512 : (j + 1) * 512], psqk[0:64, :]
                    )
                    nc.vector.tensor_copy(
                        kz[0:64, j * 512 : (j + 1) * 512], psqk[64:128, :]
                    )
                s1["qz"], s1["kz"] = qz, kz

                # v direct in [t, h] layout: stationary = xt chunk t-half,
                # moving = Wv chunk. 8 regions x 3 accumulating MMs, N=64.
                psv = psv_pool.tile([128, 512], F32, tag="psv")
                for b in range(GRP):
                    for th in range(2):
                        off = b * 128 + th * 64
                        toff = b * T + th * 128
                        for c in range(NCHUNK):
                            nc.tensor.matmul(
                                psv[:, off : off + 64],
                                lhsT=xt_sb[:, c, toff : toff + 128],
                                rhs=wv_sb[:, c, :],
                                start=(c == 0),
                                stop=(c == NCHUNK - 1),
                            )
                v4 = vaug[g1 % 2]
                dst = v4.rearrange("p b (two g) -> p b two g", two=2)[
                    :, :, :, 0:64
                ]
                src = psv[:].rearrange("p (b two f) -> p b two f", b=GRP, two=2)
                nc.vector.tensor_copy(dst, src)
                s1["v4"] = v4

                # weiT pair A (batches 0,1): K=128 zero-padded, FWL-eligible
                # 128-col stationary loads. psw [128, 384] per batch:
                #   cols 0:128   = wei[s 128:256, t 128:256] (s-half1 diag)
                #   cols 128:384 = wei[s 0:128,   t 0:256]   (s-half0 full)
                s1["psw"] = {}
                s1["p0"] = {}
                for b in (0, 1):
                    _weiT(nc, psw_pool, s1, qz, kz, b)
                for b in (0, 1):
                    _softmax(nc, p0_pool, s1, mask_sb, b)

            if 0 <= g2:
                s2 = st[g2]
                v4 = s2["v4"]
                # outT: per batch, 2 accumulating MMs (s-halves). lhsT is the
                # 128-col padded v_aug slice: M=128 (rows 65:128 of the
                # output are zeros), col 64 = ones -> row 64 = softmax denom.
                # P tiles were produced last iteration, so no PE stall here;
                # this also fills the wait for this iteration's q/k evacs.
                pso = [
                    pso_pool.tile([128, 512], F32, tag="pso", name="pso")
                    for _ in range(2)
                ]
                for b in range(GRP):
                    j, bb = divmod(b, 2)
                    nc.tensor.matmul(
                        pso[j][:, bb * T : (bb + 1) * T],
                        lhsT=v4[:, b, 0:128],
                        rhs=s2["p0"][b][:, 128:384],
                        start=True,
                        stop=False,
                    )
                    # s-half1 contributes only to t >= 128 (causal).
                    nc.tensor.matmul(
                        pso[j][:, bb * T + 128 : (bb + 1) * T],
                        lhsT=v4[:, b, 128:256],
                        rhs=s2["p0"][b][:, 0:128],
                        start=False,
                        stop=True,
                    )

                o_sb = o_pool.tile([H + 1, GRP * T], F16, tag="o")
                nc.vector.tensor_copy(o_sb[:, 0 : 2 * T], pso[0][0 : H + 1, :])
                nc.vector.tensor_copy(o_sb[:, 2 * T : 4 * T], pso[1][0 : H + 1, :])
                nc.sync.dma_start(outT[g2], o_sb[:])
                del st[g2]

            if 0 <= g1 < ng:
                s1 = st[g1]
                # weiT pair B (batches 2,3) + softmax; pair A ran before outT
                # so its exps (which gate psw buffer reuse) are already done.
                for b in (2, 3):
                    _weiT(nc, psw_pool, s1, s1["qz"], s1["kz"], b)
                for b in (2, 3):
                    _softmax(nc, p0_pool, s1, mask_sb, b)

    nc.finalize()
    return nc


def _host_inputs(x, Wq, Wk, Wv):
    B_, T_, C_ = x.shape
    assert (B_, T_, C_) == (B, T, C), (B_, T_, C_)
    # xh[g, p, c, 256*i + t] = x[4g+i, t, 128c+p]
    xh = np.ascontiguousarray(
        x.reshape(B // GRP, GRP, T, NCHUNK, 128)
        .transpose(0, 4, 3, 1, 2)
        .reshape(B // GRP, 128, NCHUNK, GRP * T)
        .astype(np.float16)
    )
    wqk_h = np.ascontiguousarray(
        np.concatenate([Wq, Wk], axis=1).reshape(NCHUNK, 128, 128).transpose(1, 0, 2),
        dtype=np.float16,
    )
    wv_h = np.ascontiguousarray(
        Wv.reshape(NCHUNK, 128, H).transpose(1, 0, 2), dtype=np.float16
    )
    tri = np.triu(np.ones((128, 128), dtype=np.float16))
    mask_h = np.ascontiguousarray(np.concatenate([tri, tri], axis=1))
    return xh, wqk_h, wv_h, mask_h


def _gather(results):
    """Concatenate per-core outT, normalize, and restore [B, T, H] fp32."""
    outT = np.concatenate(
        [results[i]["outT"] for i in range(N_CORES)], axis=0
    ).astype(np.float32)  # [B/GRP, 65, GRP*T]
    outT = outT.reshape(B // GRP, H + 1, GRP, T)
    out = outT[:, 0:H] / outT[:, H : H + 1]  # [B/GRP, H, GRP, T]
    return np.ascontiguousarray(
        out.transpose(0, 2, 3, 1).reshape(B, T, H).astype(np.float32)
    )


def kernel(x, Wq, Wk, Wv):
    x = np.asarray(x, dtype=np.float32)
    Wq = np.asarray(Wq, dtype=np.float32)
    Wk = np.asarray(Wk, dtype=np.float32)
    Wv = np.asarray(Wv, dtype=np.float32)

    xh, wqk_h, wv_h, mask_h = _host_inputs(x, Wq, Wk, Wv)

    nc = build_nc(NG)
    in_maps = [
        {
            "xt": xh[i * NG : (i + 1) * NG],
            "wqk": wqk_h,
            "wv": wv_h,
            "mask": mask_h,
        }
        for i in range(N_CORES)
    ]
    res = run_bass_kernel_spmd(nc, in_maps, list(range(N_CORES)))
    return _gather(res.results)
